# revision 1
# baseline (speedup 1.0000x reference)
"""Trainium2 Bass kernel for nn_BranchingLayer (gnn_message_passing).

Reference computation (shapes hardcoded from the spec):
  x:[786432,32] f32, global_features:[2048,16], parents_idxs:[524288] i32,
  W1:[48,128], b1:[128], W2:[128,128], b2:[128]
  parents = x[parents_idxs]                # [524288, 32], row i = (p, b)
  h  = leaky_relu(concat(parents, g[b]) @ W1 + b1, 0.01)
  proj = h @ W2 + b2 + repeat_interleave(parents, 4, -1)
  children[(p*4+br)*2048 + b, f] = proj[p*2048+b, br*32+f]
  out = concat([x, children], 0)           # [2883584, 32]

Design:
 * Shard the 256 parents over 8 cores (32/core); per-core x and output
   slices are contiguous.
 * fp16 matmuls (fp32 PE runs at 1/4 rate; fp16 has 2 more mantissa bits
   than bf16 at the same speed), fp32 PSUM accumulation.
   leaky(z) = 0.99*relu(z) + 0.01*z with the linear 0.01*z@W2 term folded
   into the residual matmul weights (host-precomputed in f64).  The
   residual (out += x) is kept ~fp32-exact by a hi/lo fp16 split of x,
   with the lo rows merged into the same K=81 residual matmul.
 * Feature-major compute: per parent/quarter, psum1[128f,512] =
   W1'^T.xt (K=49, bias via ones row), h1 = relu(psum1) (ACT, fp16),
   psum2[128j,512] = W2'^T.h1 + ER^T.xt (K=81: residual + lin + biases
   + lo-correction); DVE 32x32 block-transpose psum2 -> bt.
 * Batch columns are host-permuted: position 32c+d holds row 64d+c.
   After the 32x32 block transpose, partition 32*br+d holds rows
   64d..64d+64 of branch br contiguously -> each output DMA is 32
   descriptors x 8KB (full line rate), one per (parent, branch), on the
   otherwise-idle GPSIMD (SWDGE) ring.
"""

import numpy as np

BATCH = 2048
NPAR = 256
NF = 32
NG = 16
NBR = 4
OFF = 262144
NCORES = 8
PPC = NPAR // NCORES          # parents per core
QW = 512                      # matmul free-dim (quarter of batch)
NQ = BATCH // QW
XROWS = 81                    # 0-31 x_hi, 32-47 g_hi, 48 ones, 49-80 x_lo

_CACHE = {}


def _build_nc(ppc=PPC, reps=1):
    import concourse.bacc as bacc
    import concourse.bass as bass
    import concourse.mybir as mybir
    import concourse.tile as tile
    from contextlib import ExitStack, nullcontext

    bf = mybir.dt.float16
    f32 = mybir.dt.float32
    nc = bacc.Bacc("TRN2", target_bir_lowering=False, debug=False)

    xt_d = nc.dram_tensor("xt", [ppc, XROWS, BATCH], bf, kind="ExternalInput")
    w1_d = nc.dram_tensor("w1", [49, 128], bf, kind="ExternalInput")
    w2_d = nc.dram_tensor("w2", [128, 128], bf, kind="ExternalInput")
    er_d = nc.dram_tensor("er", [XROWS, 128], bf, kind="ExternalInput")
    out_d = nc.dram_tensor("out", [ppc * NBR * BATCH, NF], f32, kind="ExternalOutput")

    with tile.TileContext(nc) as tc, ExitStack() as ctx:
        wpool = ctx.enter_context(tc.tile_pool(name="w", bufs=1))
        xpool = ctx.enter_context(tc.tile_pool(name="x", bufs=4))
        hpool = ctx.enter_context(tc.tile_pool(name="h", bufs=8))
        btpool = ctx.enter_context(tc.tile_pool(name="bt", bufs=4))
        p1pool = ctx.enter_context(
            tc.tile_pool(name="p1", bufs=4, space=bass.MemorySpace.PSUM)
        )
        p2pool = ctx.enter_context(
            tc.tile_pool(name="p2", bufs=3, space=bass.MemorySpace.PSUM)
        )

        w1_t = wpool.tile([49, 128], bf, tag="w1")
        nc.sync.dma_start(w1_t[:], w1_d[:])
        w2_t = wpool.tile([128, 128], bf, tag="w2")
        nc.sync.dma_start(w2_t[:], w2_d[:])
        er_t = wpool.tile([XROWS, 128], bf, tag="er")
        nc.sync.dma_start(er_t[:], er_d[:])

        rep_ctx = tc.For_i(0, reps, 1) if reps > 1 else nullcontext()
        with rep_ctx:
            for pp in range(0, ppc, 2):
                pair = []
                for j in range(2):
                    xt_t = xpool.tile([XROWS, BATCH], bf, tag="xt")
                    nc.sync.dma_start(xt_t[:], xt_d[pp + j])
                    bt_t = btpool.tile([128, BATCH], f32, tag="bt")
                    pair.append((xt_t, bt_t))
                for q in range(NQ):
                    s = slice(q * QW, (q + 1) * QW)
                    for xt_t, bt_t in pair:
                        ps1 = p1pool.tile([128, QW], f32, tag="ps1")
                        nc.tensor.matmul(
                            ps1[:], w1_t[:], xt_t[:49, s], start=True, stop=True
                        )
                        h1 = hpool.tile([128, QW], bf, tag="h1")
                        nc.scalar.activation(
                            h1[:], ps1[:], mybir.ActivationFunctionType.Relu
                        )
                        ps2 = p2pool.tile([128, QW], f32, tag="ps2")
                        nc.tensor.matmul(ps2[:], w2_t[:], h1[:], start=True, stop=False)
                        nc.tensor.matmul(
                            ps2[:], er_t[:], xt_t[:, s], start=False, stop=True
                        )
                        nc.vector.transpose(bt_t[:, s], ps2[:])
                for j in range(2):
                    for br in range(NBR):
                        row0 = ((pp + j) * NBR + br) * BATCH
                        dst = out_d[row0 : row0 + BATCH, :].rearrange(
                            "(d c) f -> d (c f)", d=32
                        )
                        src = pair[j][1][32 * br : 32 * (br + 1), :]
                        nc.gpsimd.dma_start(dst, src)
    nc.compile()
    return nc


def _get_nc():
    if "nc" not in _CACHE:
        _CACHE["nc"] = _build_nc()
    return _CACHE["nc"]


def _perm_cols(a):
    """Permute the trailing batch axis: position 32c+d <- row 64d+c."""
    shp = a.shape[:-1]
    return np.ascontiguousarray(
        a.reshape(*shp, 32, 64).swapaxes(-1, -2).reshape(*shp, BATCH)
    )


def _pack_inputs(x, global_features, parents_idxs, W1, b1, W2, b2, ppc=PPC):
    """Build the per-core input maps (host-side sharding + layout)."""
    bf16 = np.float16
    x = np.asarray(x, np.float32)
    g = np.asarray(global_features, np.float32)
    idx = np.asarray(parents_idxs)
    W1 = np.asarray(W1, np.float32)
    b1 = np.asarray(b1, np.float32)
    W2 = np.asarray(W2, np.float32)
    b2 = np.asarray(b2, np.float32)

    n_rows = NPAR * BATCH
    exp = np.arange(n_rows, dtype=np.int64)
    if np.array_equal(idx, exp + OFF):
        parents = x[OFF : OFF + n_rows]
    else:
        parents = x[idx]  # general gather
    gi = idx.astype(np.int64) % BATCH
    if not np.array_equal(gi, np.tile(np.arange(BATCH, dtype=np.int64), NPAR)):
        return None

    # Feature-major per-parent x with permuted batch columns
    xf = parents.reshape(NPAR, BATCH, NF).transpose(0, 2, 1)  # [P, 32, B]
    xf = _perm_cols(xf)
    x_hi = xf.astype(bf16)
    x_lo = (xf - x_hi.astype(np.float32)).astype(bf16)
    g_hi = _perm_cols(np.ascontiguousarray(g.T)).astype(bf16)  # [16, B]

    xt = np.empty((NPAR, XROWS, BATCH), bf16)
    xt[:, :32] = x_hi
    xt[:, 32:48] = g_hi[None]
    xt[:, 48] = np.float32(1.0)
    xt[:, 49:81] = x_lo

    W1f = W1.astype(np.float64)
    W2f = W2.astype(np.float64)
    lin = 0.01 * (W1f @ W2f)  # [48, 128]
    w1 = np.concatenate([W1, b1[None]], axis=0).astype(bf16)  # [49, 128]
    w2 = (0.99 * W2f).astype(bf16)
    er = np.zeros((XROWS, 128), np.float64)
    jj = np.arange(128)
    er[jj // 4, jj] = 1.0
    er[:48] += lin
    er[48] = b2.astype(np.float64) + 0.01 * (b1.astype(np.float64) @ W2f)
    er[49 + jj // 4, jj] = 1.0
    er = er.astype(bf16)

    ncores = NPAR // ppc
    in_maps = []
    for c in range(ncores):
        in_maps.append(
            {
                "xt": xt[c * ppc : (c + 1) * ppc],
                "w1": w1,
                "w2": w2,
                "er": er,
            }
        )
    return in_maps


def _numpy_fallback(x, global_features, parents_idxs, W1, b1, W2, b2):
    x = np.asarray(x, np.float32)
    g = np.asarray(global_features, np.float32)
    idx = np.asarray(parents_idxs).astype(np.int64)
    pf = x[idx]
    pg = g[idx % BATCH]
    h = np.concatenate([pf, pg], axis=-1) @ np.asarray(W1, np.float32) + b1
    h = np.where(h > 0, h, 0.01 * h).astype(np.float32)
    proj = h @ np.asarray(W2, np.float32) + b2
    proj = proj + np.repeat(pf, NBR, axis=-1)
    m = proj.reshape(NPAR, BATCH, NF * NBR)
    m = np.swapaxes(m, 1, 2)
    m = m.reshape(NPAR * NBR, NF, BATCH)
    m = np.swapaxes(m, 1, 2)
    children = m.reshape(NPAR * NBR * BATCH, NF)
    return np.concatenate([x, children], axis=0).astype(np.float32)


def kernel(x, global_features, parents_idxs, W1, b1, W2, b2):
    in_maps = _pack_inputs(x, global_features, parents_idxs, W1, b1, W2, b2)
    if in_maps is None:
        return _numpy_fallback(x, global_features, parents_idxs, W1, b1, W2, b2)

    from concourse.bass_utils import run_bass_kernel_spmd

    nc = _get_nc()
    res = run_bass_kernel_spmd(nc, in_maps, core_ids=list(range(NCORES)))
    _CACHE["last_result"] = res

    x = np.asarray(x, np.float32)
    out = np.empty((x.shape[0] + NPAR * NBR * BATCH, NF), np.float32)
    out[: x.shape[0]] = x
    base = x.shape[0]
    per = PPC * NBR * BATCH
    for c in range(NCORES):
        out[base + c * per : base + (c + 1) * per] = res.results[c]["out"]
    return out



# revision 2
# speedup vs baseline: 1.4088x; 1.4088x over previous
"""Trainium2 Bass kernel for nn_BranchingLayer (gnn_message_passing).

Reference computation (shapes hardcoded from the spec):
  x:[786432,32] f32, global_features:[2048,16], parents_idxs:[524288] i32,
  W1:[48,128], b1:[128], W2:[128,128], b2:[128]
  parents = x[parents_idxs]                # [524288, 32], row i = (p, b)
  h  = leaky_relu(concat(parents, g[b]) @ W1 + b1, 0.01)
  proj = h @ W2 + b2 + repeat_interleave(parents, 4, -1)
  children[(p*4+br)*2048 + b, f] = proj[p*2048+b, br*32+f]
  out = concat([x, children], 0)           # [2883584, 32]

Design:
 * Shard the 256 parents over 8 cores (32/core); per-core x and output
   slices are contiguous.
 * fp16 matmuls (fp32 PE runs at 1/4 rate), fp32 PSUM accumulation.
   leaky(z) = 0.99*relu(z) + 0.01*z with the linear 0.01*z@W2 term folded
   into the residual matmul weights (host-precomputed in f64).
 * Feature-major compute: per parent/quarter, psum1[128f,512] =
   W1'^T.xt (K=49, bias via ones row), h1 = relu(psum1) (ACT, fp16),
   psum2[128j,512] = W2'^T.h1 + ER^T.xt (K=49: residual + lin + biases);
   DVE 32x32 block-transpose psum2 -> bt.
 * Batch columns are host-permuted: position 32c+d holds row 64d+c.
   After the 32x32 block transpose, partition 32*br+d of bt holds batch
   rows {64d+c} of branch br as one contiguous 8KB DRAM chunk, and the
   chunks are partition-ordered -> the whole parent is ONE 128-partition
   1MB contiguous output DMA (all 16 SDMA engines engaged) on the
   otherwise-idle GPSIMD (SWDGE) ring.
"""

import numpy as np

BATCH = 2048
NPAR = 256
NF = 32
NG = 16
NBR = 4
OFF = 262144
NCORES = 8
PPC = NPAR // NCORES          # parents per core
QW = 512                      # matmul free-dim (quarter of batch)
NQ = BATCH // QW
XROWS = 49                    # 0-31 x, 32-47 g, 48 ones

_CACHE = {}


def _build_nc(ppc=PPC, reps=1):
    import concourse.bacc as bacc
    import concourse.bass as bass
    import concourse.mybir as mybir
    import concourse.tile as tile
    from contextlib import ExitStack, nullcontext

    bf = mybir.dt.float16
    f32 = mybir.dt.float32
    nc = bacc.Bacc("TRN2", target_bir_lowering=False, debug=False)

    xt_d = nc.dram_tensor("xt", [ppc, XROWS, BATCH], bf, kind="ExternalInput")
    w1_d = nc.dram_tensor("w1", [XROWS, 128], bf, kind="ExternalInput")
    er_d = nc.dram_tensor("er", [XROWS, 128], bf, kind="ExternalInput")
    w2_d = nc.dram_tensor("w2", [128, 128], bf, kind="ExternalInput")
    out_d = nc.dram_tensor("out", [ppc, 128, BATCH], f32, kind="ExternalOutput")

    with tile.TileContext(nc) as tc, ExitStack() as ctx:
        wpool = ctx.enter_context(tc.tile_pool(name="w", bufs=1))
        xpool = ctx.enter_context(tc.tile_pool(name="x", bufs=4))
        hpool = ctx.enter_context(tc.tile_pool(name="h", bufs=8))
        btpool = ctx.enter_context(tc.tile_pool(name="bt", bufs=4))
        p1pool = ctx.enter_context(
            tc.tile_pool(name="p1", bufs=4, space=bass.MemorySpace.PSUM)
        )
        p2pool = ctx.enter_context(
            tc.tile_pool(name="p2", bufs=3, space=bass.MemorySpace.PSUM)
        )

        w1_t = wpool.tile([XROWS, 128], bf, tag="w1")
        nc.sync.dma_start(w1_t[:], w1_d[:])
        w2_t = wpool.tile([128, 128], bf, tag="w2")
        nc.sync.dma_start(w2_t[:], w2_d[:])
        er_t = wpool.tile([XROWS, 128], bf, tag="er")
        nc.sync.dma_start(er_t[:], er_d[:])

        rep_ctx = tc.For_i(0, reps, 1) if reps > 1 else nullcontext()
        with rep_ctx:
            for pp in range(ppc):
                xt_t = xpool.tile([XROWS, BATCH], bf, tag="xt")
                nc.sync.dma_start(xt_t[:], xt_d[pp])
                bt_t = btpool.tile([128, BATCH], f32, tag="bt")
                for q in range(NQ):
                    s = slice(q * QW, (q + 1) * QW)
                    ps1 = p1pool.tile([128, QW], f32, tag="ps1")
                    nc.tensor.matmul(
                        ps1[:], w1_t[:], xt_t[:, s], start=True, stop=True
                    )
                    h1 = hpool.tile([128, QW], bf, tag="h1")
                    nc.scalar.activation(
                        h1[:], ps1[:], mybir.ActivationFunctionType.Relu
                    )
                    ps2 = p2pool.tile([128, QW], f32, tag="ps2")
                    nc.tensor.matmul(ps2[:], w2_t[:], h1[:], start=True, stop=False)
                    nc.tensor.matmul(
                        ps2[:], er_t[:], xt_t[:, s], start=False, stop=True
                    )
                    nc.vector.transpose(bt_t[:, s], ps2[:])
                nc.gpsimd.dma_start(out_d[pp], bt_t[:])
    nc.compile()
    return nc


def _get_nc():
    if "nc" not in _CACHE:
        _CACHE["nc"] = _build_nc()
    return _CACHE["nc"]


def _perm_cols(a):
    """Permute the trailing batch axis: position 32c+d <- row 64d+c."""
    shp = a.shape[:-1]
    return np.ascontiguousarray(
        a.reshape(*shp, 32, 64).swapaxes(-1, -2).reshape(*shp, BATCH)
    )


def _pack_inputs(x, global_features, parents_idxs, W1, b1, W2, b2, ppc=PPC):
    """Build the per-core input maps (host-side sharding + layout)."""
    bf16 = np.float16
    x = np.asarray(x, np.float32)
    g = np.asarray(global_features, np.float32)
    idx = np.asarray(parents_idxs)
    W1 = np.asarray(W1, np.float32)
    b1 = np.asarray(b1, np.float32)
    W2 = np.asarray(W2, np.float32)
    b2 = np.asarray(b2, np.float32)

    n_rows = NPAR * BATCH
    exp = np.arange(n_rows, dtype=np.int64)
    if np.array_equal(idx, exp + OFF):
        parents = x[OFF : OFF + n_rows]
    else:
        parents = x[idx]  # general gather
    gi = idx.astype(np.int64) % BATCH
    if not np.array_equal(gi, np.tile(np.arange(BATCH, dtype=np.int64), NPAR)):
        return None

    # Feature-major per-parent x with permuted batch columns
    xf = parents.reshape(NPAR, BATCH, NF).transpose(0, 2, 1)  # [P, 32, B]
    xf = _perm_cols(xf)
    g_hi = _perm_cols(np.ascontiguousarray(g.T)).astype(bf16)  # [16, B]

    xt = np.empty((NPAR, XROWS, BATCH), bf16)
    xt[:, :32] = xf.astype(bf16)
    xt[:, 32:48] = g_hi[None]
    xt[:, 48] = np.float32(1.0)

    W1f = W1.astype(np.float64)
    W2f = W2.astype(np.float64)
    lin = 0.01 * (W1f @ W2f)  # [48, 128]
    w1 = np.zeros((XROWS, 128), np.float32)
    w1[:48] = W1
    w1[48] = b1
    w1 = w1.astype(bf16)
    w2 = (0.99 * W2f).astype(bf16)
    er = np.zeros((XROWS, 128), np.float64)
    jj = np.arange(128)
    er[jj // 4, jj] = 1.0
    er[:48] += lin
    er[48] = b2.astype(np.float64) + 0.01 * (b1.astype(np.float64) @ W2f)
    er = er.astype(bf16)

    ncores = NPAR // ppc
    in_maps = []
    for c in range(ncores):
        in_maps.append(
            {
                "xt": xt[c * ppc : (c + 1) * ppc],
                "w1": w1,
                "w2": w2,
                "er": er,
            }
        )
    return in_maps


def _numpy_fallback(x, global_features, parents_idxs, W1, b1, W2, b2):
    x = np.asarray(x, np.float32)
    g = np.asarray(global_features, np.float32)
    idx = np.asarray(parents_idxs).astype(np.int64)
    pf = x[idx]
    pg = g[idx % BATCH]
    h = np.concatenate([pf, pg], axis=-1) @ np.asarray(W1, np.float32) + b1
    h = np.where(h > 0, h, 0.01 * h).astype(np.float32)
    proj = h @ np.asarray(W2, np.float32) + b2
    proj = proj + np.repeat(pf, NBR, axis=-1)
    m = proj.reshape(NPAR, BATCH, NF * NBR)
    m = np.swapaxes(m, 1, 2)
    m = m.reshape(NPAR * NBR, NF, BATCH)
    m = np.swapaxes(m, 1, 2)
    children = m.reshape(NPAR * NBR * BATCH, NF)
    return np.concatenate([x, children], axis=0).astype(np.float32)


def kernel(x, global_features, parents_idxs, W1, b1, W2, b2):
    in_maps = _pack_inputs(x, global_features, parents_idxs, W1, b1, W2, b2)
    if in_maps is None:
        return _numpy_fallback(x, global_features, parents_idxs, W1, b1, W2, b2)

    from concourse.bass_utils import run_bass_kernel_spmd

    nc = _get_nc()
    res = run_bass_kernel_spmd(nc, in_maps, core_ids=list(range(NCORES)))
    _CACHE["last_result"] = res

    x = np.asarray(x, np.float32)
    out = np.empty((x.shape[0] + NPAR * NBR * BATCH, NF), np.float32)
    out[: x.shape[0]] = x
    base = x.shape[0]
    per = PPC * NBR * BATCH
    for c in range(NCORES):
        out[base + c * per : base + (c + 1) * per] = res.results[c]["out"].reshape(
            per, NF
        )
    return out


# revision 5
# speedup vs baseline: 1.5969x; 1.1335x over previous
"""Trainium2 Bass kernel for nn_BranchingLayer (gnn_message_passing).

Reference computation (shapes hardcoded from the spec):
  x:[786432,32] f32, global_features:[2048,16], parents_idxs:[524288] i32,
  W1:[48,128], b1:[128], W2:[128,128], b2:[128]
  parents = x[parents_idxs]                # [524288, 32], row i = (p, b)
  h  = leaky_relu(concat(parents, g[b]) @ W1 + b1, 0.01)
  proj = h @ W2 + b2 + repeat_interleave(parents, 4, -1)
  children[(p*4+br)*2048 + b, f] = proj[p*2048+b, br*32+f]
  out = concat([x, children], 0)           # [2883584, 32]

Design:
 * Shard the 256 parents over 8 cores (32/core); per-core x and output
   slices are contiguous.
 * fp16 matmuls (fp32 PE runs at 1/4 rate), fp32 PSUM accumulation.
   leaky(z) = 0.99*relu(z) + 0.01*z with the linear 0.01*z@W2 term folded
   into the residual matmul weights (host-precomputed in f64).
 * Feature-major compute: per parent/quarter, psum1[128f,512] =
   W1'^T.xt (K=49, bias via ones row), h1 = relu(psum1) (ACT, fp16),
   psum2[128j,512] = W2'^T.h1 + ER^T.xt (K=49: residual + lin + biases);
   DVE 32x32 block-transpose psum2 -> bt.
 * Batch columns are host-permuted: position 32c+d holds row 64d+c.
   After the 32x32 block transpose, partition 32*br+d of bt holds batch
   rows {64d+c} of branch br as one contiguous 8KB DRAM chunk, and the
   chunks are partition-ordered -> the whole parent is ONE 128-partition
   1MB contiguous output DMA (all 16 SDMA engines engaged) on the
   otherwise-idle GPSIMD (SWDGE) ring.
"""

import numpy as np

BATCH = 2048
NPAR = 256
NF = 32
NG = 16
NBR = 4
OFF = 262144
NCORES = 8
PPC = NPAR // NCORES          # parents per core
QW = 512                      # matmul free-dim (quarter of batch)
NQ = BATCH // QW
XROWS = 49                    # 0-31 x, 32-47 g, 48 ones

_CACHE = {}


def _build_nc(ppc=PPC, reps=1):
    import concourse.bacc as bacc
    import concourse.bass as bass
    import concourse.mybir as mybir
    import concourse.tile as tile
    from contextlib import ExitStack, nullcontext

    bf = mybir.dt.float16
    f32 = mybir.dt.float32
    nc = bacc.Bacc("TRN2", target_bir_lowering=False, debug=False)

    xt_d = nc.dram_tensor("xt", [ppc, XROWS, BATCH], bf, kind="ExternalInput")
    w1_d = nc.dram_tensor("w1", [XROWS, 128], bf, kind="ExternalInput")
    er_d = nc.dram_tensor("er", [XROWS, 128], bf, kind="ExternalInput")
    w2_d = nc.dram_tensor("w2", [128, 128], bf, kind="ExternalInput")
    out_d = nc.dram_tensor("out", [ppc, 128, BATCH], f32, kind="ExternalOutput")

    with tile.TileContext(nc) as tc, ExitStack() as ctx:
        wpool = ctx.enter_context(tc.tile_pool(name="w", bufs=1))
        xpool = ctx.enter_context(tc.tile_pool(name="x", bufs=3))
        hpool = ctx.enter_context(tc.tile_pool(name="h", bufs=8))
        btpool = ctx.enter_context(tc.tile_pool(name="bt", bufs=4))
        p1pool = ctx.enter_context(
            tc.tile_pool(name="p1", bufs=4, space=bass.MemorySpace.PSUM)
        )
        p2pool = ctx.enter_context(
            tc.tile_pool(name="p2", bufs=3, space=bass.MemorySpace.PSUM)
        )

        # w1/er duplicated at partition rows 0-48 and 64-112 so both the
        # even-parent (rows 0-48) and odd-parent (rows 64-112) xt slices
        # see a stationary operand at their own base partition.
        w1_t = wpool.tile([64 + XROWS, 128], bf, tag="w1")
        nc.sync.dma_start(w1_t[:XROWS], w1_d[:])
        nc.sync.dma_start(w1_t[64 : 64 + XROWS], w1_d[:])
        w2_t = wpool.tile([128, 128], bf, tag="w2")
        nc.sync.dma_start(w2_t[:], w2_d[:])
        er_t = wpool.tile([64 + XROWS, 128], bf, tag="er")
        nc.sync.dma_start(er_t[:XROWS], er_d[:])
        nc.sync.dma_start(er_t[64 : 64 + XROWS], er_d[:])

        rep_ctx = tc.For_i(0, reps, 1) if reps > 1 else nullcontext()
        with rep_ctx:
            # Pipelined schedule: mm1+relu for parent p+1 are emitted
            # between parent p's mm2/transpose groups, so the PE queue
            # never waits on ACT.  xt is loaded in 2-parent pair tiles at
            # partition rows 0-48 / 64-112 (spreads the input DMA over
            # both the even and odd SDMA engine groups).
            ROW2 = 64
            xt_tiles = {}   # pair index -> tile
            ps1_t = {}      # (parent, q) -> psum tile
            h1_t = {}       # (parent, q) -> sbuf fp16 tile

            def load_pair(j):
                if j * 2 >= ppc:
                    return
                t = xpool.tile([128, BATCH], bf, tag="xt")
                nc.sync.dma_start(t[:XROWS], xt_d[2 * j])
                if 2 * j + 1 < ppc:
                    nc.sync.dma_start(t[ROW2 : ROW2 + XROWS], xt_d[2 * j + 1])
                xt_tiles[j] = t

            def xt_rows(p):
                t = xt_tiles[p // 2]
                r0 = ROW2 * (p % 2)
                return t[r0 : r0 + XROWS]

            def w_rows(t, p):
                r0 = ROW2 * (p % 2)
                return t[r0 : r0 + XROWS]

            def mm1(p, q):
                s = slice(q * QW, (q + 1) * QW)
                ps1 = p1pool.tile([128, QW], f32, tag="ps1")
                nc.tensor.matmul(
                    ps1[:], w_rows(w1_t, p), xt_rows(p)[:, s], start=True, stop=True
                )
                h1 = hpool.tile([128, QW], bf, tag="h1")
                nc.scalar.activation(h1[:], ps1[:], mybir.ActivationFunctionType.Relu)
                ps1_t[(p, q)] = ps1
                h1_t[(p, q)] = h1

            def mm2(p, q, bt_t):
                s = slice(q * QW, (q + 1) * QW)
                ps2 = p2pool.tile([128, QW], f32, tag="ps2")
                nc.tensor.matmul(ps2[:], w2_t[:], h1_t.pop((p, q))[:], start=True, stop=False)
                nc.tensor.matmul(
                    ps2[:], w_rows(er_t, p), xt_rows(p)[:, s], start=False, stop=True
                )
                nc.vector.transpose(bt_t[:, s], ps2[:])
                ps1_t.pop((p, q))

            load_pair(0)
            load_pair(1)
            for q in range(NQ):
                mm1(0, q)
            for p in range(ppc):
                if p % 2 == 0 and p + 4 < ppc:
                    load_pair(p // 2 + 2)
                bt_t = btpool.tile([128, BATCH], f32, tag="bt")
                for half in range(2):
                    for q in (0, 1) if half == 0 else (2, 3):
                        mm2(p, q, bt_t)
                    if p + 1 < ppc:
                        for q in (0, 1) if half == 0 else (2, 3):
                            mm1(p + 1, q)
                nc.gpsimd.dma_start(out_d[p], bt_t[:])
    nc.compile()
    return nc


def _get_nc():
    if "nc" not in _CACHE:
        _CACHE["nc"] = _build_nc()
    return _CACHE["nc"]


def _perm_cols(a):
    """Permute the trailing batch axis: position 32c+d <- row 64d+c."""
    shp = a.shape[:-1]
    return np.ascontiguousarray(
        a.reshape(*shp, 32, 64).swapaxes(-1, -2).reshape(*shp, BATCH)
    )


def _pack_inputs(x, global_features, parents_idxs, W1, b1, W2, b2, ppc=PPC):
    """Build the per-core input maps (host-side sharding + layout)."""
    bf16 = np.float16
    x = np.asarray(x, np.float32)
    g = np.asarray(global_features, np.float32)
    idx = np.asarray(parents_idxs)
    W1 = np.asarray(W1, np.float32)
    b1 = np.asarray(b1, np.float32)
    W2 = np.asarray(W2, np.float32)
    b2 = np.asarray(b2, np.float32)

    n_rows = NPAR * BATCH
    exp = np.arange(n_rows, dtype=np.int64)
    if np.array_equal(idx, exp + OFF):
        parents = x[OFF : OFF + n_rows]
    else:
        parents = x[idx]  # general gather
    gi = idx.astype(np.int64) % BATCH
    if not np.array_equal(gi, np.tile(np.arange(BATCH, dtype=np.int64), NPAR)):
        return None

    # Feature-major per-parent x with permuted batch columns
    xf = parents.reshape(NPAR, BATCH, NF).transpose(0, 2, 1)  # [P, 32, B]
    xf = _perm_cols(xf)
    g_hi = _perm_cols(np.ascontiguousarray(g.T)).astype(bf16)  # [16, B]

    xt = np.empty((NPAR, XROWS, BATCH), bf16)
    xt[:, :32] = xf.astype(bf16)
    xt[:, 32:48] = g_hi[None]
    xt[:, 48] = np.float32(1.0)

    W1f = W1.astype(np.float64)
    W2f = W2.astype(np.float64)
    lin = 0.01 * (W1f @ W2f)  # [48, 128]
    w1 = np.zeros((XROWS, 128), np.float32)
    w1[:48] = W1
    w1[48] = b1
    w1 = w1.astype(bf16)
    w2 = (0.99 * W2f).astype(bf16)
    er = np.zeros((XROWS, 128), np.float64)
    jj = np.arange(128)
    er[jj // 4, jj] = 1.0
    er[:48] += lin
    er[48] = b2.astype(np.float64) + 0.01 * (b1.astype(np.float64) @ W2f)
    er = er.astype(bf16)

    ncores = NPAR // ppc
    in_maps = []
    for c in range(ncores):
        in_maps.append(
            {
                "xt": xt[c * ppc : (c + 1) * ppc],
                "w1": w1,
                "w2": w2,
                "er": er,
            }
        )
    return in_maps


def _numpy_fallback(x, global_features, parents_idxs, W1, b1, W2, b2):
    x = np.asarray(x, np.float32)
    g = np.asarray(global_features, np.float32)
    idx = np.asarray(parents_idxs).astype(np.int64)
    pf = x[idx]
    pg = g[idx % BATCH]
    h = np.concatenate([pf, pg], axis=-1) @ np.asarray(W1, np.float32) + b1
    h = np.where(h > 0, h, 0.01 * h).astype(np.float32)
    proj = h @ np.asarray(W2, np.float32) + b2
    proj = proj + np.repeat(pf, NBR, axis=-1)
    m = proj.reshape(NPAR, BATCH, NF * NBR)
    m = np.swapaxes(m, 1, 2)
    m = m.reshape(NPAR * NBR, NF, BATCH)
    m = np.swapaxes(m, 1, 2)
    children = m.reshape(NPAR * NBR * BATCH, NF)
    return np.concatenate([x, children], axis=0).astype(np.float32)


def kernel(x, global_features, parents_idxs, W1, b1, W2, b2):
    in_maps = _pack_inputs(x, global_features, parents_idxs, W1, b1, W2, b2)
    if in_maps is None:
        return _numpy_fallback(x, global_features, parents_idxs, W1, b1, W2, b2)

    from concourse.bass_utils import run_bass_kernel_spmd

    nc = _get_nc()
    res = run_bass_kernel_spmd(nc, in_maps, core_ids=list(range(NCORES)))
    _CACHE["last_result"] = res

    x = np.asarray(x, np.float32)
    out = np.empty((x.shape[0] + NPAR * NBR * BATCH, NF), np.float32)
    out[: x.shape[0]] = x
    base = x.shape[0]
    per = PPC * NBR * BATCH
    for c in range(NCORES):
        out[base + c * per : base + (c + 1) * per] = res.results[c]["out"].reshape(
            per, NF
        )
    return out


# revision 6
# speedup vs baseline: 2.1047x; 1.3180x over previous
"""Trainium2 Bass kernel for nn_BranchingLayer (gnn_message_passing).

Reference computation (shapes hardcoded from the spec):
  x:[786432,32] f32, global_features:[2048,16], parents_idxs:[524288] i32,
  W1:[48,128], b1:[128], W2:[128,128], b2:[128]
  parents = x[parents_idxs]                # [524288, 32], row i = (p, b)
  h  = leaky_relu(concat(parents, g[b]) @ W1 + b1, 0.01)
  proj = h @ W2 + b2 + repeat_interleave(parents, 4, -1)
  children[(p*4+br)*2048 + b, f] = proj[p*2048+b, br*32+f]
  out = concat([x, children], 0)           # [2883584, 32]

Design:
 * Shard the 256 parents over 8 cores (32/core); per-core x and output
   slices are contiguous.
 * fp16 matmuls (fp32 PE runs at 1/4 rate), fp32 PSUM accumulation.
   leaky(z) = 0.99*relu(z) + 0.01*z with the linear 0.01*z@W2 term folded
   into the residual matmul weights (host-precomputed in f64).
 * K=128 everywhere: K<128 matmuls measure ~3x slower per free-column on
   HW, so two parents share one [128, 2048] xt tile (A rows 0-48, B rows
   49-97; rows 98-127 zeroed once) and every matmul streams the full 128
   partitions.  Parent selection happens in the stationary operand: wA
   has the weights in rows 0-48 with rows 49+ zero, wB in rows 49-97 --
   the other parent's data multiplies zeros.  Stationary swaps are free
   (double-buffered weight load).
 * Feature-major compute: per parent/quarter, psum1[128f,512] =
   w^T.xt (bias via ones row), h1 = relu(psum1) (ACT, fp16),
   psum2[128j,512] = W2^T.h1 + er^T.xt (residual + lin + biases);
   DVE 32x32 block-transpose psum2 -> bt.  mm1+relu of parent p+1 are
   emitted between parent p's mm2 groups so PE never waits on ACT.
 * Batch columns are host-permuted: position 32c+d holds row 64d+c.
   After the 32x32 block transpose, partition 32*br+d of bt holds batch
   rows {64d+c} of branch br as one contiguous 8KB DRAM chunk, and the
   chunks are partition-ordered -> the whole parent is ONE 128-partition
   1MB contiguous output DMA (all 16 SDMA engines engaged) on the
   otherwise-idle GPSIMD (SWDGE) ring.
"""

import numpy as np

BATCH = 2048
NPAR = 256
NF = 32
NG = 16
NBR = 4
OFF = 262144
NCORES = 8
PPC = NPAR // NCORES          # parents per core
QW = 512                      # matmul free-dim (quarter of batch)
NQ = BATCH // QW
XROWS = 49                    # 0-31 x, 32-47 g, 48 ones

_CACHE = {}


def _build_nc(ppc=PPC, reps=1):
    import concourse.bacc as bacc
    import concourse.bass as bass
    import concourse.mybir as mybir
    import concourse.tile as tile
    from contextlib import ExitStack, nullcontext

    bf = mybir.dt.float16
    f32 = mybir.dt.float32
    nc = bacc.Bacc("TRN2", target_bir_lowering=False, debug=False)

    npair = ppc // 2
    xt_d = nc.dram_tensor("xt", [npair, 2 * XROWS, BATCH], bf, kind="ExternalInput")
    w1e_d = nc.dram_tensor("w1e", [128, 128], bf, kind="ExternalInput")
    w1o_d = nc.dram_tensor("w1o", [128, 128], bf, kind="ExternalInput")
    ere_d = nc.dram_tensor("ere", [128, 128], bf, kind="ExternalInput")
    ero_d = nc.dram_tensor("ero", [128, 128], bf, kind="ExternalInput")
    w2_d = nc.dram_tensor("w2", [128, 128], bf, kind="ExternalInput")
    out_d = nc.dram_tensor("out", [ppc, 128, BATCH], f32, kind="ExternalOutput")

    with tile.TileContext(nc) as tc, ExitStack() as ctx:
        wpool = ctx.enter_context(tc.tile_pool(name="w", bufs=1))
        hpool = ctx.enter_context(tc.tile_pool(name="h", bufs=8))
        btpool = ctx.enter_context(tc.tile_pool(name="bt", bufs=4))
        p1pool = ctx.enter_context(
            tc.tile_pool(name="p1", bufs=4, space=bass.MemorySpace.PSUM)
        )
        p2pool = ctx.enter_context(
            tc.tile_pool(name="p2", bufs=3, space=bass.MemorySpace.PSUM)
        )

        w_t = {}
        for name, dram in (
            ("w1e", w1e_d), ("w1o", w1o_d), ("ere", ere_d), ("ero", ero_d),
            ("w2", w2_d),
        ):
            t = wpool.tile([128, 128], bf, tag=name)
            nc.sync.dma_start(t[:], dram[:])
            w_t[name] = t

        NXT = 3
        xt_static = []
        for i in range(NXT):
            t = wpool.tile([128, BATCH], bf, tag=f"xts{i}")
            nc.vector.memset(t[:], 0.0)
            xt_static.append(t)

        rep_ctx = tc.For_i(0, reps, 1) if reps > 1 else nullcontext()
        with rep_ctx:
            h1_t = {}

            def load_pair(j):
                if j >= npair:
                    return
                nc.sync.dma_start(xt_static[j % NXT][: 2 * XROWS, :], xt_d[j])

            def xt_tile(p):
                return xt_static[(p // 2) % NXT]

            def mm1(p, q):
                s = slice(q * QW, (q + 1) * QW)
                w1 = w_t["w1e"] if p % 2 == 0 else w_t["w1o"]
                ps1 = p1pool.tile([128, QW], f32, tag="ps1")
                nc.tensor.matmul(ps1[:], w1[:], xt_tile(p)[:, s], start=True, stop=True)
                h1 = hpool.tile([128, QW], bf, tag="h1")
                nc.scalar.activation(h1[:], ps1[:], mybir.ActivationFunctionType.Relu)
                h1_t[(p, q)] = h1

            def mm2(p, q, bt_t):
                s = slice(q * QW, (q + 1) * QW)
                er = w_t["ere"] if p % 2 == 0 else w_t["ero"]
                ps2 = p2pool.tile([128, QW], f32, tag="ps2")
                nc.tensor.matmul(
                    ps2[:], w_t["w2"][:], h1_t.pop((p, q))[:], start=True, stop=False
                )
                nc.tensor.matmul(ps2[:], er[:], xt_tile(p)[:, s], start=False, stop=True)
                nc.vector.transpose(bt_t[:, s], ps2[:])

            load_pair(0)
            load_pair(1)
            for q in range(NQ):
                mm1(0, q)
            for p in range(ppc):
                if p % 2 == 0:
                    load_pair(p // 2 + 2)
                bt_t = btpool.tile([128, BATCH], f32, tag="bt")
                for half in range(2):
                    for q in (0, 1) if half == 0 else (2, 3):
                        mm2(p, q, bt_t)
                    if p + 1 < ppc:
                        for q in (0, 1) if half == 0 else (2, 3):
                            mm1(p + 1, q)
                nc.gpsimd.dma_start(out_d[p], bt_t[:])
    nc.compile()
    return nc


def _get_nc():
    if "nc" not in _CACHE:
        _CACHE["nc"] = _build_nc()
    return _CACHE["nc"]


def _perm_cols(a):
    """Permute the trailing batch axis: position 32c+d <- row 64d+c."""
    shp = a.shape[:-1]
    return np.ascontiguousarray(
        a.reshape(*shp, 32, 64).swapaxes(-1, -2).reshape(*shp, BATCH)
    )


def _pack_inputs(x, global_features, parents_idxs, W1, b1, W2, b2, ppc=PPC):
    """Build the per-core input maps (host-side sharding + layout)."""
    bf16 = np.float16
    x = np.asarray(x, np.float32)
    g = np.asarray(global_features, np.float32)
    idx = np.asarray(parents_idxs)
    W1 = np.asarray(W1, np.float32)
    b1 = np.asarray(b1, np.float32)
    W2 = np.asarray(W2, np.float32)
    b2 = np.asarray(b2, np.float32)

    n_rows = NPAR * BATCH
    exp = np.arange(n_rows, dtype=np.int64)
    if np.array_equal(idx, exp + OFF):
        parents = x[OFF : OFF + n_rows]
    else:
        parents = x[idx]  # general gather
    gi = idx.astype(np.int64) % BATCH
    if not np.array_equal(gi, np.tile(np.arange(BATCH, dtype=np.int64), NPAR)):
        return None

    # Feature-major per-parent x with permuted batch columns
    xf = parents.reshape(NPAR, BATCH, NF).transpose(0, 2, 1)  # [P, 32, B]
    xf = _perm_cols(xf)
    g_hi = _perm_cols(np.ascontiguousarray(g.T)).astype(bf16)  # [16, B]

    xt = np.empty((NPAR, XROWS, BATCH), bf16)
    xt[:, :32] = xf.astype(bf16)
    xt[:, 32:48] = g_hi[None]
    xt[:, 48] = np.float32(1.0)
    # pair layout: [npair_total, 98, B] -- parent 2j rows 0-48, 2j+1 rows 49-97
    xtp = xt.reshape(NPAR // 2, 2 * XROWS, BATCH)

    W1f = W1.astype(np.float64)
    W2f = W2.astype(np.float64)
    lin = 0.01 * (W1f @ W2f)  # [48, 128]
    w1 = np.zeros((XROWS, 128), np.float32)
    w1[:48] = W1
    w1[48] = b1
    er = np.zeros((XROWS, 128), np.float64)
    jj = np.arange(128)
    er[jj // 4, jj] = 1.0
    er[:48] += lin
    er[48] = b2.astype(np.float64) + 0.01 * (b1.astype(np.float64) @ W2f)

    def pad128(m, row0):
        out = np.zeros((128, 128), np.float32)
        out[row0 : row0 + XROWS] = m
        return out.astype(bf16)

    w1e = pad128(w1, 0)
    w1o = pad128(w1, XROWS)
    ere = pad128(er, 0)
    ero = pad128(er, XROWS)
    w2 = (0.99 * W2f).astype(bf16)

    ncores = NPAR // ppc
    npair = ppc // 2
    in_maps = []
    for c in range(ncores):
        in_maps.append(
            {
                "xt": xtp[c * npair : (c + 1) * npair],
                "w1e": w1e,
                "w1o": w1o,
                "ere": ere,
                "ero": ero,
                "w2": w2,
            }
        )
    return in_maps


def _numpy_fallback(x, global_features, parents_idxs, W1, b1, W2, b2):
    x = np.asarray(x, np.float32)
    g = np.asarray(global_features, np.float32)
    idx = np.asarray(parents_idxs).astype(np.int64)
    pf = x[idx]
    pg = g[idx % BATCH]
    h = np.concatenate([pf, pg], axis=-1) @ np.asarray(W1, np.float32) + b1
    h = np.where(h > 0, h, 0.01 * h).astype(np.float32)
    proj = h @ np.asarray(W2, np.float32) + b2
    proj = proj + np.repeat(pf, NBR, axis=-1)
    m = proj.reshape(NPAR, BATCH, NF * NBR)
    m = np.swapaxes(m, 1, 2)
    m = m.reshape(NPAR * NBR, NF, BATCH)
    m = np.swapaxes(m, 1, 2)
    children = m.reshape(NPAR * NBR * BATCH, NF)
    return np.concatenate([x, children], axis=0).astype(np.float32)


def kernel(x, global_features, parents_idxs, W1, b1, W2, b2):
    in_maps = _pack_inputs(x, global_features, parents_idxs, W1, b1, W2, b2)
    if in_maps is None:
        return _numpy_fallback(x, global_features, parents_idxs, W1, b1, W2, b2)

    from concourse.bass_utils import run_bass_kernel_spmd

    nc = _get_nc()
    res = run_bass_kernel_spmd(nc, in_maps, core_ids=list(range(NCORES)))
    _CACHE["last_result"] = res

    x = np.asarray(x, np.float32)
    out = np.empty((x.shape[0] + NPAR * NBR * BATCH, NF), np.float32)
    out[: x.shape[0]] = x
    base = x.shape[0]
    per = PPC * NBR * BATCH
    for c in range(NCORES):
        out[base + c * per : base + (c + 1) * per] = res.results[c]["out"].reshape(
            per, NF
        )
    return out


# revision 9
# speedup vs baseline: 2.4430x; 1.1608x over previous
"""Trainium2 Bass kernel for nn_BranchingLayer (gnn_message_passing).

Reference computation (shapes hardcoded from the spec):
  x:[786432,32] f32, global_features:[2048,16], parents_idxs:[524288] i32,
  W1:[48,128], b1:[128], W2:[128,128], b2:[128]
  parents = x[parents_idxs]                # [524288, 32], row i = (p, b)
  h  = leaky_relu(concat(parents, g[b]) @ W1 + b1, 0.01)
  proj = h @ W2 + b2 + repeat_interleave(parents, 4, -1)
  children[(p*4+br)*2048 + b, f] = proj[p*2048+b, br*32+f]
  out = concat([x, children], 0)           # [2883584, 32]

Design:
 * Shard the 256 parents over 8 cores (32/core); per-core x and output
   slices are contiguous.
 * fp16 matmuls (fp32 PE runs at 1/4 rate), fp32 PSUM accumulation.
   leaky(z) = 0.99*relu(z) + 0.01*z with the linear 0.01*z@W2 term folded
   into the residual matmul weights (host-precomputed in f64).
 * K=128 everywhere: K<128 matmuls measure ~3x slower per free-column on
   HW, so two parents share one [128, 2048] xt tile (A rows 0-48, B rows
   49-97; rows 98-127 zeroed once) and every matmul streams the full 128
   partitions.  Parent selection happens in the stationary operand: wA
   has the weights in rows 0-48 with rows 49+ zero, wB in rows 49-97 --
   the other parent's data multiplies zeros.  Stationary swaps are free
   (double-buffered weight load).
 * Feature-major compute: per parent/quarter, psum1[128f,512] =
   w^T.xt (bias via ones row), h1 = relu(psum1) (ACT, fp16),
   psum2[128j,512] = W2^T.h1 + er^T.xt (residual + lin + biases);
   DVE 32x32 block-transpose psum2 -> bt (f32).  The output DMA casts
   f32->fp16 in flight (SWDGE CME cast): the 2e-2 rel-err budget easily
   covers fp16 children, and halving the output bytes halves the
   dominant HBM write traffic; the host upcasts when assembling.
   mm1+relu of parent p+1 are emitted between parent p's mm2 groups so
   PE never waits on ACT.
 * Batch columns are host-permuted: position 32c+d holds row 64d+c.
   After the 32x32 block transpose, partition 32*br+d of bt holds batch
   rows {64d+c} of branch br as one contiguous 8KB DRAM chunk, and the
   chunks are partition-ordered -> the whole parent is ONE 128-partition
   1MB contiguous output DMA (all 16 SDMA engines engaged) on the
   otherwise-idle GPSIMD (SWDGE) ring.
"""

import numpy as np

BATCH = 2048
NPAR = 256
NF = 32
NG = 16
NBR = 4
OFF = 262144
NCORES = 8
PPC = NPAR // NCORES          # parents per core
QW = 512                      # matmul free-dim (quarter of batch)
NQ = BATCH // QW
XROWS = 49                    # 0-31 x, 32-47 g, 48 ones

_CACHE = {}


def _build_nc(ppc=PPC, reps=1):
    import concourse.bacc as bacc
    import concourse.bass as bass
    import concourse.mybir as mybir
    import concourse.tile as tile
    from contextlib import ExitStack, nullcontext

    bf = mybir.dt.float16
    f32 = mybir.dt.float32
    nc = bacc.Bacc("TRN2", target_bir_lowering=False, debug=False)

    npair = ppc // 2
    xt_d = nc.dram_tensor("xt", [npair, 2 * XROWS, BATCH], bf, kind="ExternalInput")
    w1e_d = nc.dram_tensor("w1e", [128, 128], bf, kind="ExternalInput")
    w1o_d = nc.dram_tensor("w1o", [128, 128], bf, kind="ExternalInput")
    ere_d = nc.dram_tensor("ere", [128, 128], bf, kind="ExternalInput")
    ero_d = nc.dram_tensor("ero", [128, 128], bf, kind="ExternalInput")
    w2_d = nc.dram_tensor("w2", [128, 128], bf, kind="ExternalInput")
    out_d = nc.dram_tensor("out", [ppc, 128, BATCH], bf, kind="ExternalOutput")

    with tile.TileContext(nc) as tc, ExitStack() as ctx:
        wpool = ctx.enter_context(tc.tile_pool(name="w", bufs=1))
        hpool = ctx.enter_context(tc.tile_pool(name="h", bufs=8))
        btpool = ctx.enter_context(tc.tile_pool(name="bt", bufs=4))
        p1pool = ctx.enter_context(
            tc.tile_pool(name="p1", bufs=4, space=bass.MemorySpace.PSUM)
        )
        p2pool = ctx.enter_context(
            tc.tile_pool(name="p2", bufs=3, space=bass.MemorySpace.PSUM)
        )

        w_t = {}
        for name, dram in (
            ("w1e", w1e_d), ("w1o", w1o_d), ("ere", ere_d), ("ero", ero_d),
            ("w2", w2_d),
        ):
            t = wpool.tile([128, 128], bf, tag=name)
            nc.sync.dma_start(t[:], dram[:])
            w_t[name] = t

        NXT = 3
        xt_static = []
        for i in range(NXT):
            t = wpool.tile([128, BATCH], bf, tag=f"xts{i}")
            nc.vector.memset(t[:], 0.0)
            xt_static.append(t)

        rep_ctx = tc.For_i(0, reps, 1) if reps > 1 else nullcontext()
        with rep_ctx:
            h1_t = {}

            def load_pair(j):
                if j >= npair:
                    return
                nc.sync.dma_start(xt_static[j % NXT][: 2 * XROWS, :], xt_d[j])

            def xt_tile(p):
                return xt_static[(p // 2) % NXT]

            def mm1(p, q):
                s = slice(q * QW, (q + 1) * QW)
                w1 = w_t["w1e"] if p % 2 == 0 else w_t["w1o"]
                ps1 = p1pool.tile([128, QW], f32, tag="ps1")
                nc.tensor.matmul(ps1[:], w1[:], xt_tile(p)[:, s], start=True, stop=True)
                h1 = hpool.tile([128, QW], bf, tag="h1")
                nc.scalar.activation(h1[:], ps1[:], mybir.ActivationFunctionType.Relu)
                h1_t[(p, q)] = h1

            def mm2(p, q, bt_t):
                s = slice(q * QW, (q + 1) * QW)
                er = w_t["ere"] if p % 2 == 0 else w_t["ero"]
                ps2 = p2pool.tile([128, QW], f32, tag="ps2")
                nc.tensor.matmul(
                    ps2[:], w_t["w2"][:], h1_t.pop((p, q))[:], start=True, stop=False
                )
                nc.tensor.matmul(ps2[:], er[:], xt_tile(p)[:, s], start=False, stop=True)
                nc.vector.transpose(bt_t[:, s], ps2[:])

            load_pair(0)
            load_pair(1)
            for q in range(NQ):
                mm1(0, q)
            for p in range(ppc):
                if p % 2 == 0:
                    load_pair(p // 2 + 2)
                bt_t = btpool.tile([128, BATCH], f32, tag="bt")
                for half in range(2):
                    for q in (0, 1) if half == 0 else (2, 3):
                        mm2(p, q, bt_t)
                    if p + 1 < ppc:
                        for q in (0, 1) if half == 0 else (2, 3):
                            mm1(p + 1, q)
                nc.gpsimd.dma_start(out_d[p], bt_t[:])
    nc.compile()
    return nc


def _get_nc():
    if "nc" not in _CACHE:
        _CACHE["nc"] = _build_nc()
    return _CACHE["nc"]


def _perm_cols(a):
    """Permute the trailing batch axis: position 32c+d <- row 64d+c."""
    shp = a.shape[:-1]
    return np.ascontiguousarray(
        a.reshape(*shp, 32, 64).swapaxes(-1, -2).reshape(*shp, BATCH)
    )


def _pack_inputs(x, global_features, parents_idxs, W1, b1, W2, b2, ppc=PPC):
    """Build the per-core input maps (host-side sharding + layout)."""
    bf16 = np.float16
    x = np.asarray(x, np.float32)
    g = np.asarray(global_features, np.float32)
    idx = np.asarray(parents_idxs)
    W1 = np.asarray(W1, np.float32)
    b1 = np.asarray(b1, np.float32)
    W2 = np.asarray(W2, np.float32)
    b2 = np.asarray(b2, np.float32)

    n_rows = NPAR * BATCH
    exp = np.arange(n_rows, dtype=np.int64)
    if np.array_equal(idx, exp + OFF):
        parents = x[OFF : OFF + n_rows]
    else:
        parents = x[idx]  # general gather
    gi = idx.astype(np.int64) % BATCH
    if not np.array_equal(gi, np.tile(np.arange(BATCH, dtype=np.int64), NPAR)):
        return None

    # Feature-major per-parent x with permuted batch columns
    xf = parents.reshape(NPAR, BATCH, NF).transpose(0, 2, 1)  # [P, 32, B]
    xf = _perm_cols(xf)
    g_hi = _perm_cols(np.ascontiguousarray(g.T)).astype(bf16)  # [16, B]

    xt = np.empty((NPAR, XROWS, BATCH), bf16)
    xt[:, :32] = xf.astype(bf16)
    xt[:, 32:48] = g_hi[None]
    xt[:, 48] = np.float32(1.0)
    # pair layout: [npair_total, 98, B] -- parent 2j rows 0-48, 2j+1 rows 49-97
    xtp = xt.reshape(NPAR // 2, 2 * XROWS, BATCH)

    W1f = W1.astype(np.float64)
    W2f = W2.astype(np.float64)
    lin = 0.01 * (W1f @ W2f)  # [48, 128]
    w1 = np.zeros((XROWS, 128), np.float32)
    w1[:48] = W1
    w1[48] = b1
    er = np.zeros((XROWS, 128), np.float64)
    jj = np.arange(128)
    er[jj // 4, jj] = 1.0
    er[:48] += lin
    er[48] = b2.astype(np.float64) + 0.01 * (b1.astype(np.float64) @ W2f)

    def pad128(m, row0):
        out = np.zeros((128, 128), np.float32)
        out[row0 : row0 + XROWS] = m
        return out.astype(bf16)

    w1e = pad128(w1, 0)
    w1o = pad128(w1, XROWS)
    ere = pad128(er, 0)
    ero = pad128(er, XROWS)
    w2 = (0.99 * W2f).astype(bf16)

    ncores = NPAR // ppc
    npair = ppc // 2
    in_maps = []
    for c in range(ncores):
        in_maps.append(
            {
                "xt": xtp[c * npair : (c + 1) * npair],
                "w1e": w1e,
                "w1o": w1o,
                "ere": ere,
                "ero": ero,
                "w2": w2,
            }
        )
    return in_maps


def _numpy_fallback(x, global_features, parents_idxs, W1, b1, W2, b2):
    x = np.asarray(x, np.float32)
    g = np.asarray(global_features, np.float32)
    idx = np.asarray(parents_idxs).astype(np.int64)
    pf = x[idx]
    pg = g[idx % BATCH]
    h = np.concatenate([pf, pg], axis=-1) @ np.asarray(W1, np.float32) + b1
    h = np.where(h > 0, h, 0.01 * h).astype(np.float32)
    proj = h @ np.asarray(W2, np.float32) + b2
    proj = proj + np.repeat(pf, NBR, axis=-1)
    m = proj.reshape(NPAR, BATCH, NF * NBR)
    m = np.swapaxes(m, 1, 2)
    m = m.reshape(NPAR * NBR, NF, BATCH)
    m = np.swapaxes(m, 1, 2)
    children = m.reshape(NPAR * NBR * BATCH, NF)
    return np.concatenate([x, children], axis=0).astype(np.float32)


def kernel(x, global_features, parents_idxs, W1, b1, W2, b2):
    in_maps = _pack_inputs(x, global_features, parents_idxs, W1, b1, W2, b2)
    if in_maps is None:
        return _numpy_fallback(x, global_features, parents_idxs, W1, b1, W2, b2)

    from concourse.bass_utils import run_bass_kernel_spmd

    nc = _get_nc()
    res = run_bass_kernel_spmd(nc, in_maps, core_ids=list(range(NCORES)))
    _CACHE["last_result"] = res

    x = np.asarray(x, np.float32)
    out = np.empty((x.shape[0] + NPAR * NBR * BATCH, NF), np.float32)
    out[: x.shape[0]] = x
    base = x.shape[0]
    per = PPC * NBR * BATCH
    for c in range(NCORES):
        out[base + c * per : base + (c + 1) * per] = res.results[c]["out"].reshape(
            per, NF
        )
    return out
